# revision 7
# baseline (speedup 1.0000x reference)
"""Biaffine edge attention on 8 Trainium2 NeuronCores (fp16, PE-bound schedule).

Math (per batch b):
    out[i,o] = head[i,:] @ U @ dep[o,:] + head[i,:]@wh + dep[o,:]@wd + b
with head/dep [S=2048, D=256], U [D,D], edge_W = [wh | wd] (each [D]).

Sharding: pure data-parallel over batch B=8 -> one batch per core,
U / edge_W / edge_b replicated. No collectives.

Per-core kernel (matmul operands fp16, PSUM accum f32, fp16 out stores
upcast to f32 on host; max rel err ~2e-3 vs the 2e-2 gate):
    ATf[e,i] = sum_d U[d,e] * headT[d,i] + wd[e]      (dep-side rank-1 term
               ds[o] rides the e-contraction for free)
    hs[i]    = sum_d head[i,d] * wh[d]  + b           (fused DVE ttr-reduce,
               bias as the reduction init)
    out[i,o] = sum_e ATf[e,i] * depT[e,o]  + hs[i]    (f32 epilogue, fp16 out)

At fp16 the kernel is PE-bound (~34us of 1-cycle/row matmul vs ~30us of
DMA). Schedule notes (engine queues are in-order; ACT has a depth-0 exec
queue, so nothing speculative goes there):
  - all 4 dep groups load first, then the 4 head groups, back-to-back on
    the SP queue; row-major out rows start right after depT completes;
  - consts ride the Pool (SWDGE) queue: eyeu first (eye gates the very
    first transpose), and a dummy activation preloads ACT's table during
    the load phase;
  - GPSIMD cannot touch PSUM (hw rule), so PSUM->SBUF copies alternate
    ACT/DVE (2+2 per out row), transpose collects alternate DVE/ACT, atf
    bias-copies on ACT, hs on DVE; Pool keeps the const DMAs + broadcast.
"""

import numpy as np

import concourse.bass as bass
import concourse.tile as tile
from concourse import bacc, mybir
from concourse.bass_utils import run_bass_kernel_spmd

B, S, D = 8, 2048, 256
P = 128          # partitions
OC = 512         # matmul output free-dim chunk (one PSUM bank of fp32)
GB = 4           # row-blocks per input load group
NG = S // (P * GB)   # 4 load groups per input
NI = S // P      # 16 row blocks
NO = S // OC     # 4 output column chunks
ND = D // P      # 2 contraction chunks
F32 = mybir.dt.float32
F16 = mybir.dt.float16

Ident = mybir.ActivationFunctionType.Identity
Mult = mybir.AluOpType.mult
Add = mybir.AluOpType.add


def build_nc(reps=1):
    """reps>1 wraps the body in a HW For_i loop -- used only for timing."""
    nc = bacc.Bacc("TRN2", target_bir_lowering=False, debug=False, num_devices=B)

    head_d = nc.dram_tensor("head", [S, D], F16, kind="ExternalInput")
    dep_d = nc.dram_tensor("dep", [S, D], F16, kind="ExternalInput")
    # eye [128] | u0 [256] | u1 [256] packed on the free dim
    eyeu_d = nc.dram_tensor("eyeu", [P, P + D * ND], F16, kind="ExternalInput")
    # b128 | wdT0 | wdT1 packed
    cw_d = nc.dram_tensor("cw", [P, 1 + ND], F32, kind="ExternalInput")
    whb_d = nc.dram_tensor("wh_b", [1, D], F16, kind="ExternalInput")
    out_d = nc.dram_tensor("out", [S, S], F16, kind="ExternalOutput")

    with tile.TileContext(nc) as tc:
        with (
            tc.tile_pool(name="const", bufs=1) as cpool,
            tc.tile_pool(name="persist", bufs=1) as ppool,
            tc.tile_pool(name="dstage", bufs=4) as dstage,
            tc.tile_pool(name="hstage", bufs=4) as hstage,
            tc.tile_pool(name="ttrp", bufs=2) as ttrp,
            tc.tile_pool(name="outbuf", bufs=6) as outbuf,
            tc.tile_pool(name="ps_t", bufs=3, space=bass.MemorySpace.PSUM) as ps_t,
            tc.tile_pool(name="ps_mm", bufs=5, space=bass.MemorySpace.PSUM) as ps_mm,
        ):
            # ---- persistent SBUF tensors ----
            headT = [ppool.tile([P, S], F16, name=f"headT{dc}", tag=f"headT{dc}")
                     for dc in range(ND)]
            depT = [ppool.tile([P, S], F16, name=f"depT{dc}", tag=f"depT{dc}")
                    for dc in range(ND)]
            atf = [ppool.tile([P, S], F16, name=f"atf{eb}", tag=f"atf{eb}")
                   for eb in range(ND)]
            hs_colb = ppool.tile([P, NI], F32, name="hs_colb", tag="hs_colb")
            hs_col = ppool.tile([P, NI], F32, name="hs_col", tag="hs_col")

            def load_group(src_dram, g, pool):
                # [128, GB*D]: free = (block j, d); one DMA, 3D src pattern
                nat = pool.tile([P, GB * D], F16, name="nat", tag="nat")
                src = src_dram[g * GB * P:(g + 1) * GB * P, :]
                src3 = src.rearrange("(j p) d -> p j d", p=P)
                nc.sync.dma_start(nat[:].rearrange("p (j d) -> p j d", d=D), src3)
                return nat

            def transpose_group(nat, dstT, g, eng_off, eye):
                # 8 PE transposes -> two [128,512] PSUM collect tiles -> 2
                # copies, spread over DVE and Pool
                for dc in range(ND):
                    pst = ps_t.tile([P, GB * P], F16, name="pst", tag="pst")
                    for j in range(GB):
                        nc.tensor.transpose(
                            pst[:, j * P:(j + 1) * P],
                            nat[:, j * D + dc * P: j * D + dc * P + P],
                            eye[:],
                        )
                    dst = dstT[dc][:, g * GB * P:(g + 1) * GB * P]
                    if (g * ND + dc + eng_off) % 2 == 0:
                        nc.vector.tensor_copy(dst, pst[:])
                    else:
                        nc.scalar.copy(dst, pst[:])

            def body():
                # ---- consts on the Pool (SWDGE) queue, eyeu first ----
                eyeu = cpool.tile([P, P + D * ND], F16, name="eyeu", tag="eyeu")
                nc.gpsimd.dma_start(eyeu[:], eyeu_d[:])
                cw = cpool.tile([P, 1 + ND], F32, name="cw", tag="cw")
                nc.gpsimd.dma_start(cw[:], cw_d[:])
                whb = cpool.tile([1, D], F16, name="whb", tag="whb")
                nc.gpsimd.dma_start(whb[:], whb_d[:])
                eye = eyeu[:, 0:P]
                u_sb = [eyeu[:, P + dc * D: P + (dc + 1) * D] for dc in range(ND)]
                b128 = cw[:, 0:1]
                wdT = cw[:, 1:1 + ND]

                # warm ACT's activation table while loads run
                scratch = cpool.tile([1, 1], F32, name="scr", tag="scr")
                nc.vector.memset(scratch[:], 0.0)
                nc.scalar.activation(scratch[:], scratch[:], Ident)

                # ---- input loads: all dep, then all head, on SP queue ----
                nat_d = [load_group(dep_d, g, dstage) for g in range(NG)]
                nat_h = [load_group(head_d, g, hstage) for g in range(NG)]

                # wh broadcast [1,D] -> [128,D] on the Pool engine
                wh_rep = cpool.tile([P, D], F16, name="wh_rep", tag="wh_rep")
                nc.gpsimd.partition_broadcast(wh_rep[:], whb[:])

                def head_group(g):
                    # transpose + ATf columns + hs for head group g
                    transpose_group(nat_h[g], headT, g, 1, eye)
                    for eb in range(ND):
                        pa = ps_mm.tile([P, OC], F32, name="psmm", tag="psmm")
                        for dc in range(ND):
                            nc.tensor.matmul(
                                pa[:],
                                u_sb[dc][:, eb * P:(eb + 1) * P],
                                headT[dc][:, g * OC:(g + 1) * OC],
                                start=(dc == 0),
                                stop=(dc == ND - 1),
                            )
                        if eb == 0:
                            nc.scalar.activation(
                                atf[eb][:, g * OC:(g + 1) * OC], pa[:], Ident,
                                bias=wdT[:, eb:eb + 1],
                            )
                        else:
                            nc.vector.tensor_scalar_add(
                                atf[eb][:, g * OC:(g + 1) * OC], pa[:],
                                wdT[:, eb:eb + 1],
                            )
                    # hs: Pool mul (SBUF only), DVE blockwise reduce, ACT bias
                    ttr = ttrp.tile([P, GB * D], F32, name="ttr", tag="ttr")
                    nc.gpsimd.tensor_mul(
                        ttr[:].rearrange("p (j d) -> p j d", d=D),
                        nat_h[g][:].rearrange("p (j d) -> p j d", d=D),
                        wh_rep[:].rearrange("p (j d) -> p j d", j=1)
                        .broadcast_to([P, GB, D]))
                    nc.vector.reduce_sum(
                        hs_col[:, g * GB:(g + 1) * GB],
                        ttr[:].rearrange("p (j d) -> p j d", d=D),
                        axis=mybir.AxisListType.X,
                    )
                    nc.vector.tensor_scalar_add(
                        hs_colb[:, g * GB:(g + 1) * GB],
                        hs_col[:, g * GB:(g + 1) * GB], b128,
                    )

                def out_row(ib):
                    # two independent half-row staging tiles; each stored by
                    # its own DMA as soon as its two epilogue copies land
                    for h in range(2):
                        ot = outbuf.tile([P, S // 2], F16, name="ot", tag="ot")
                        for k in range(2):
                            oc = 2 * h + k
                            po = ps_mm.tile([P, OC], F32, name="psmm",
                                            tag="psmm")
                            for eb in range(ND):
                                nc.tensor.matmul(
                                    po[:],
                                    atf[eb][:, ib * P:(ib + 1) * P],
                                    depT[eb][:, oc * OC:(oc + 1) * OC],
                                    start=(eb == 0),
                                    stop=(eb == ND - 1),
                                )
                            dst = ot[:, k * OC:(k + 1) * OC]
                            if (ib + oc) % 2 == 0:
                                nc.scalar.activation(
                                    dst, po[:], Ident,
                                    bias=hs_colb[:, ib:ib + 1])
                            else:
                                nc.vector.tensor_scalar_add(
                                    dst, po[:], hs_colb[:, ib:ib + 1])
                        nc.sync.dma_start(
                            out_d[ib * P:(ib + 1) * P,
                                  h * 2 * OC:(h + 1) * 2 * OC], ot[:])

                # ---- compute, trailing the load stream ----
                for g in range(NG):
                    transpose_group(nat_d[g], depT, g, 0, eye)
                head_group(0)
                for g in range(NG):
                    for ib in range(g * GB, (g + 1) * GB):
                        out_row(ib)
                        if ib == 0:
                            head_group(1)
                        if ib == g * GB + 2 and g + 2 < NG:
                            head_group(g + 2)

            if reps > 1:
                with tc.For_i(0, reps, 1):
                    body()
            else:
                body()

    nc.finalize()
    return nc


_NC_CACHE = {}


def _get_nc(reps=1):
    if reps not in _NC_CACHE:
        _NC_CACHE[reps] = build_nc(reps)
    return _NC_CACHE[reps]


def make_in_maps(head, dep, edge_U, edge_W, edge_b):
    head = np.ascontiguousarray(np.asarray(head, np.float32).astype(np.float16))
    dep = np.ascontiguousarray(np.asarray(dep, np.float32).astype(np.float16))
    u = np.asarray(edge_U, np.float32).astype(np.float16)
    w = np.asarray(edge_W, dtype=np.float32).reshape(-1)
    wh, wd = w[:D], w[D:]
    eyeu = np.concatenate(
        [np.eye(P, dtype=np.float16)]
        + [u[dc * P:(dc + 1) * P, :] for dc in range(ND)], axis=1)
    eyeu = np.ascontiguousarray(eyeu)
    cw = np.empty((P, 1 + ND), np.float32)
    cw[:, 0] = float(np.asarray(edge_b).reshape(-1)[0])
    cw[:, 1:] = wd.reshape(ND, P).T
    whb = np.ascontiguousarray(wh[None, :].astype(np.float16))
    return [
        {"head": head[b], "dep": dep[b], "eyeu": eyeu, "cw": cw, "wh_b": whb}
        for b in range(B)
    ]


def kernel(head, dep, edge_U, edge_W, edge_b):
    nc = _get_nc()
    in_maps = make_in_maps(head, dep, edge_U, edge_W, edge_b)
    last_err = None
    for _ in range(3):  # transient device errors happen on this shared env
        try:
            res = run_bass_kernel_spmd(nc, in_maps, core_ids=list(range(B)))
            break
        except Exception as e:  # noqa: BLE001
            last_err = e
    else:
        raise last_err
    return np.stack(
        [res.results[b]["out"].astype(np.float32) for b in range(B)], axis=0)
